# revision 41
# baseline (speedup 1.0000x reference)
"""Masked causal self-attention on 8 trn2 NeuronCores (v2).

Problem: x[4,4096,1024] fp32; q/k/v = x @ W{q,k,v}.T (D=64);
out = softmax(causal(q k^T / 8)) v   -> [4, 4096, 64].

Sharding: core = (batch, parity).  Core (b,p) owns the alternating
128-row blocks {2i+p} of batch b (2048 q rows) and builds k/v for all
4096 rows.

Geometry (v2): global rows are grouped in 1024-row groups j=0..3.
Host chunk order per core: chunk 2j = the core's OWN 512 rows of group
j (global blocks 8j+p, 8j+2+p, 8j+4+p, 8j+6+p), chunk 2j+1 = the OTHER
parity's 512 rows.  kv sequence positions are chunk-major (no
permutation): pos 8j+t = chunk (2j + t//4)'s block (t%4).  q superblock
s (own rows 512s..512s+512) = exactly chunk 2s, so sup s's qT needs ONE
chunk and its first attention pair needs only chunk 2s's kv.

Boundary masks for group s against sup s's 4 q blocks (q block t at
cols 128t): own kv block t': visible cols >= 128t', diag tri at t==t';
other kv block t': for p=1 visible cols >= 128t' (full at t==t'), for
p=0 visible cols >= 128(t'+1).  Pairing: sup0 uses own-own/oth-oth
pairs (chunk0-only first pair); sups>=1 use mixed pairs (own t', oth
t') at c0=128t' with the baseline-style [tri | parity-flat] mask.

Changes vs the 116us baseline (measured ~85us, i.e. ~1.35x):
  - unnormalized [oT | sums] PSUM block per superblock is copied to
    SBUF and DMA'd out raw; the host does the divide + transpose
    (device tail shrinks from ~10us of transpose/reciprocal/scale
    chains to one copy + one DMA).
  - DMA priority discipline: the DMA engines serve active queues
    round-robin, so transfer ORDER (not queue choice) decides arrival.
    All x chunks go single-file on the sync queue in need-order;
    weights/masks/ident overlap only the x0 prefix on scalar/gpsimd.
  - sup0's first scores gate on ONE chunk (own/other chunk split per
    1024-row group); later chunks' projections are injected as
    per-pair fillers one superblock ahead of need.
  - per-chunk PSUM parking pool separate from the scores ring (a
    shared pool deadlocks via buf rotation + the in-order PE queue).
  - merged kvT SBUF tile ([kT; vT], chunk-major columns): one
    [128,512] PSUM->SBUF copy per chunk; q proj only for even chunks
    (512 cols, 2x perf mode for <=65-row stationaries).
  - fp8 paths dropped: measured ~6.5% output error previously (fp8
    noise on q/k/v/exp does NOT average down in softmax-attention).

Known headroom: scalar-engine exp stream is the ~36us floor; PE busy
~66us incl. kv projection duplicated across parity pairs.  A pair
ReduceScatter kv exchange (kernel_cc.py) is numerically exact but
measured UNPROFITABLE: ~10us fixed overhead per collective op (4 ops =
47.9us cc_op_time >> the 6.8us of PE saved); only a sub-us remote_dma
SBUF-to-SBUF path could make the exchange pay, and the Tile scheduler
cannot order consumers of remotely-written SBUF.  Startup DMA ramp
~13us and framework drain epilogue ~4.5us are the other fixed costs.
"""

import sys

sys.path.insert(0, "/opt/trn_rl_repo")

import numpy as np

B, S, E, D = 4, 4096, 1024, 64
P = 128
NCH = 8                  # x chunks of 512 rows (even=own, odd=other)
NSUP = 4                 # q superblocks, 512 own q rows each
OWN = S // 2             # own q rows per core
NPOS = S // P            # 32 kv positions (chunk-major)

_prog_cache = {}


def _build_program():
    import concourse.mybir as mybir
    from concourse import bacc, tile

    f32 = mybir.dt.float32
    bf16 = mybir.dt.bfloat16

    nc = bacc.Bacc("TRN2", target_bir_lowering=False, debug=False, num_devices=8)
    x_d = nc.dram_tensor("x2", [P, NCH * 8 * 512], bf16, kind="ExternalInput")
    wkv_d = nc.dram_tensor("wkv", [P, 8 * 128], bf16, kind="ExternalInput")
    wq_d = nc.dram_tensor("wq", [P, 8 * 64], bf16, kind="ExternalInput")
    # const block: [ident8(128) | mown(512) | moth(512) | mmix(256)]
    const_d = nc.dram_tensor("cst", [P, 1408], bf16, kind="ExternalInput")
    y_d = nc.dram_tensor("y", [NSUP * 65, 512], f32, kind="ExternalOutput")

    with tile.TileContext(nc) as tc:
        with (
            tc.tile_pool(name="const", bufs=1) as constp,
            tc.tile_pool(name="xin", bufs=NCH) as xin,
            tc.tile_pool(name="work", bufs=3) as work,
            tc.tile_pool(name="expp", bufs=6) as expp,
            tc.tile_pool(name="ps_s", bufs=2, space="PSUM") as ps_s,
            tc.tile_pool(name="park", bufs=2, space="PSUM") as park,
            tc.tile_pool(name="aux", bufs=1, space="PSUM") as aux,
            tc.tile_pool(name="ps_po", bufs=1, space="PSUM") as ps_po,
        ):
            ident8 = constp.tile([P, P], bf16, tag="ident8")
            # masks: [blk, 640] = [mown(256) | moth(256) | mmix(128)] per blk
            masks = constp.tile([P, 2, 640], bf16, tag="masks")
            mown = masks[:, :, 0:256]
            moth = masks[:, :, 256:512]
            mmix = masks[:, :, 512:640]
            wkv_sb = constp.tile([P, 8, 128], bf16, tag="wkv")
            wq_sb = constp.tile([P, 8, 64], bf16, tag="wq")
            # merged k^T (rows 0:64) and v^T (rows 64:128), chunk-major cols
            kvT_sb = constp.tile([P, S], bf16, tag="kvT")
            qT_sb = constp.tile([64, OWN], bf16, tag="qT")
            # v natural layout + ones col per kv position
            vOnes = constp.tile([P, NPOS, 65], bf16, tag="vOnes")

            # ---- DMA prefetch (all queues; gpsimd has lowest latency) ----
            def prefetch_all():
                # DMA engines serve active queues round-robin, so transfer
                # ORDER (not queue choice) decides arrival: x chunks go
                # single-file on sync in need-order; weights on scalar and
                # ident+masks on gpsimd overlap only the x0 prefix.
                g, sy, sc = nc.gpsimd, nc.sync, nc.scalar
                cr = const_d.ap()
                wr = wkv_d.ap().rearrange("p (c m) -> p c m", c=8)
                # chunk 0 is host-stored as two 256-row halves, each
                # [c, 256]-major, so the first attention emission gates on
                # only 0.5MB of x + half the chunk-0 projection
                x0 = x_d.ap()[:, 0:4096].rearrange("p (h c n) -> p h c n", h=2, c=8)
                x1 = x_d.ap()[:, 4096:8192].rearrange("p (c n) -> p c n", c=8)

                sc.dma_start(wkv_sb[:, 0:4], wr[:, 0:4])
                g.dma_start(ident8[:], cr[:, 0:128])
                sy.dma_start(x0t[:, 0, 2:5], x0[:, 0, 2:5])
                g.dma_start(x0t[:, 0, 0:2], x0[:, 0, 0:2])
                sc.dma_start(wkv_sb[:, 4:8], wr[:, 4:8])
                sy.dma_start(x0t[:, 0, 5:8], x0[:, 0, 5:8])
                sc.dma_start(
                    wq_sb[:], wq_d.ap().rearrange("p (c m) -> p c m", c=8)
                )
                sy.dma_start(x0t[:, 1, 0:4], x0[:, 1, 0:4])
                g.dma_start(
                    masks[:], cr[:, 128:1408].rearrange("p (k c) -> p k c", k=2)
                )
                sy.dma_start(x0t[:, 1, 4:8], x0[:, 1, 4:8])
                sy.dma_start(_xt(1)[:, 0:4], x1[:, 0:4])
                sy.dma_start(_xt(1)[:, 4:8], x1[:, 4:8])
                for it in range(2, NCH):
                    src = x_d.ap()[:, it * 4096 : (it + 1) * 4096].rearrange(
                        "p (c n) -> p c n", c=8
                    )
                    sy.dma_start(_xt(it)[:], src)
                nc.vector.memset(vOnes[:, :, 64], 1.0)

            x_tiles = {}
            x0t = xin.tile([P, 2, 8, 256], bf16, tag="xn", name="xn_0")

            def _xt(it):
                if it not in x_tiles:
                    x_tiles[it] = xin.tile([P, 8, 512], bf16, tag="xn", name=f"xn_{it}")
                return x_tiles[it]

            def warmup(n):
                # ramp the PE p-state while x streams in
                wt = aux.tile([P, P], bf16, tag="aux", name="warm")
                for _ in range(n):
                    nc.tensor.transpose(wt[:], ident8[:], ident8[:])

            # ---- projections ----
            # per-chunk PSUM parking (1 bank each, 2-deep: chunk c's bank is
            # reused by chunk c+2, whose kv mm is emitted after chunk c's
            # copies -- no cross-sup rotation hazards)
            kv_psum = {}
            q_psum = {}

            def proj_kv_mm(it):
                kv_psum[it] = park.tile([P, 512], f32, tag="park", name=f"pkv_{it}")
                xn = x_tiles[it]
                for c in range(8):
                    nc.tensor.matmul(
                        kv_psum[it][:], wkv_sb[:, c], xn[:, c],
                        start=(c == 0), stop=(c == 7),
                    )

            def proj_q_mm(it):
                # even chunks only: all 512 own rows of sup it//2
                q_psum[it] = aux.tile([64, 512], f32, tag="aux", name=f"pq_{it}")
                xn = x_tiles[it]
                for c in range(8):
                    nc.tensor.matmul(
                        q_psum[it][:], wq_sb[:, c], xn[:, c],
                        start=(c == 0), stop=(c == 7),
                    )

            def proj_copies(it, engine=None):
                """[k|v]^T copy for chunk it (PSUM -> SBUF)."""
                pkv = kv_psum.pop(it)
                r0 = it * 512
                if engine is not None:
                    engine.copy(kvT_sb[:, r0 : r0 + 512], pkv[:])
                else:
                    nc.vector.tensor_scalar_mul(
                        kvT_sb[:, r0 : r0 + 512], pkv[:], 1.0
                    )

            def proj_q_copy(it):
                s = it // 2
                nc.vector.tensor_scalar_mul(
                    qT_sb[:, s * 512 : (s + 1) * 512], q_psum.pop(it)[:], 1.0
                )

            def proj_chunk0_half(h):
                """kv+q proj, kvT and qT copies for chunk 0's 256-row half
                h (its own PSUM column range; first half gates emission 0)."""
                if h == 0:
                    kv_psum[0] = park.tile([P, 512], f32, tag="park", name="pkv_0")
                    q_psum[0] = aux.tile([64, 512], f32, tag="aux", name="pq_0")
                cs = slice(h * 256, h * 256 + 256)
                for c in range(8):
                    nc.tensor.matmul(
                        kv_psum[0][:, cs], wkv_sb[:, c], x0t[:, h, c],
                        start=(c == 0), stop=(c == 7),
                    )
                for c in range(8):
                    nc.tensor.matmul(
                        q_psum[0][:, cs], wq_sb[:, c], x0t[:, h, c],
                        start=(c == 0), stop=(c == 7),
                    )
                nc.scalar.copy(kvT_sb[:, cs], kv_psum[0][:, cs])
                nc.vector.tensor_scalar_mul(qT_sb[:, cs], q_psum[0][:, cs], 1.0)

            def proj_vtr(it):
                """v transposes + vOnes fill for chunk it.  Returns the
                deferred vOnes copy (DVE) so PE/DVE interleave."""
                r0 = it * 512
                pvt = aux.tile([P, 4, 64], bf16, tag="aux", name=f"pvt_{it}")
                for b in range(4):
                    nc.tensor.transpose(
                        pvt[:, b, :],
                        kvT_sb[64:128, r0 + b * 128 : r0 + (b + 1) * 128],
                        ident8[64:128, 64:128],
                    )
                def vcp():
                    nc.vector.tensor_copy(
                        vOnes[:, it * 4 : it * 4 + 4, 0:64], pvt[:]
                    )
                return vcp

            # ---- attention for one superblock ----
            # fillers: dict pair_idx -> [fns], run after that pair's exp
            def attend_sup(s, fillers, carry=None):
                from collections import deque

                qT_s = qT_sb[:, s * 512 : (s + 1) * 512]
                # emission list: (posA, posB, c0, c1, st, mask, mask_c0, mask_w)
                # st marks the first writer of po[:, c0:c1] (AV start flag)
                pairs = []
                for j in range(s):
                    for u in range(4):
                        pairs.append(
                            (8 * j + 2 * u, 8 * j + 2 * u + 1, 0, 512,
                             j == 0 and u == 0, None, 0, 0)
                        )
                if s == 0:
                    # pair (0,1) split at col 256 so emission 0 gates on
                    # chunk 0's first 256-row half only
                    pairs.append((0, 1, 0, 256, True, mown, 0, 256))
                    pairs.append((0, 1, 256, 512, False, None, 0, 0))
                    pairs.append((2, 3, 256, 512, False, mown, 256, 256))
                    pairs.append((4, 5, 0, 512, False, moth, 0, 256))
                    pairs.append((6, 7, 256, 512, False, moth, 256, 256))
                else:
                    for t in range(4):
                        pairs.append(
                            (8 * s + t, 8 * s + 4 + t, 128 * t, 512, False,
                             mmix, 128 * t, 128)
                        )
                npair = len(pairs)
                pobox = []

                def emit_av(pr, posA, posB, c0, c1, st, expT):
                    if not pobox:
                        pobox.append(
                            ps_po.tile([65, 512], f32, tag="po", name=f"po_{s}")
                        )
                    po = pobox[0]
                    for j, pos in enumerate((posA, posB)):
                        nc.tensor.matmul(
                            po[:, c0:c1], vOnes[:, pos, :], expT[:, j, c0:c1],
                            start=(st and j == 0),
                            stop=(pr == npair - 1 and j == 1),
                        )

                pend = deque()
                for pr, (posA, posB, c0, c1, st, mask, mc0, mw) in enumerate(pairs):
                    ps2 = ps_s.tile([P, 2, 512], f32, tag="sc")
                    for j, pos in enumerate((posA, posB)):
                        nc.tensor.matmul(
                            ps2[:, j, c0:c1],
                            kvT_sb[0:64, pos * 128 : pos * 128 + 128],
                            qT_s[:, c0:c1],
                            start=True, stop=True,
                        )
                    if mask is not None:
                        nc.vector.tensor_tensor(
                            ps2[:, :, mc0 : mc0 + mw], ps2[:, :, mc0 : mc0 + mw],
                            mask, mybir.AluOpType.add,
                        )
                    expT = expp.tile([P, 2, 512], bf16, tag="expT")
                    nc.scalar.activation(
                        expT[:, :, c0:c1], ps2[:, :, c0:c1],
                        mybir.ActivationFunctionType.Exp,
                    )
                    av0, av1 = c0, c1
                    if st and (c0 > 0 or c1 < 512):
                        # a start AV must cover the whole po bank (psum acc
                        # groups are bank-granular): zero-fill outside the
                        # exp window so the out-of-window columns init to 0
                        if c0 > 0:
                            nc.vector.memset(expT[:, :, 0:c0], 0.0)
                        if c1 < 512:
                            nc.vector.memset(expT[:, :, c1:512], 0.0)
                        av0, av1 = 0, 512
                    pend.append((pr, posA, posB, av0, av1, st, expT))
                    if pr == 0 and carry:
                        carry[0]()
                    if pr == 2 and carry and len(carry) > 1:
                        carry[1]()
                    if len(pend) > 3:
                        emit_av(*pend.popleft())
                    for fn in fillers.get(pr, ()):
                        fn()

                def flush_av():
                    while pend:
                        emit_av(*pend.popleft())

                def ship():
                    o_ac = work.tile([65, 512], f32, tag="oac", name=f"oac_{s}")
                    nc.vector.tensor_copy(o_ac[:], pobox[0][:])
                    nc.sync.dma_start(
                        y_d.ap()[s * 65 : (s + 1) * 65, :], o_ac[:]
                    )

                return [flush_av, ship]

            # ---- driver ----
            # sup s's ONLY start gate is qT(chunk 2s); its boundary pairs
            # (last 4) additionally need kT/vOnes of chunks 2s and 2s+1.
            # So: q-proj(2s+2)+qT copy runs EARLY inside sup s (removing
            # the qT chain from the sup transition), while the kv work of
            # chunks 2s+2/2s+3 is deferred into sup s+1's early pairs
            # where the exp stream already has backlog.
            prefetch_all()
            warmup(14)
            proj_chunk0_half(0)

            def ch_kv(c):
                return lambda: proj_kv_mm(c)

            def ch_q(c):
                return lambda: (proj_q_mm(c), proj_q_copy(c))

            def ch_fin(c):
                def go():
                    proj_copies(c)
                    proj_vtr(c)()
                    x_tiles.pop(c)
                return go

            fill = {
                0: {0: [lambda: proj_chunk0_half(1)],
                    1: [lambda: proj_vtr(0)(), ch_kv(1)],
                    2: [ch_fin(1), ch_q(2)]},
                1: {0: [ch_kv(2)], 1: [ch_fin(2), ch_q(4)],
                    2: [ch_kv(3)], 3: [ch_fin(3)]},
                2: {0: [ch_kv(4)], 1: [ch_fin(4), ch_q(6)],
                    2: [ch_kv(5)], 3: [ch_fin(5)]},
                3: {0: [ch_kv(6)], 1: [ch_fin(6)],
                    2: [ch_kv(7)], 3: [ch_fin(7)]},
            }
            carry = None
            for s in range(NSUP):
                flush = attend_sup(s, fill[s], carry)
                carry = flush
            for c in carry:
                c()

    nc.compile()
    return nc


def _host_inputs(x, Wq, Wk, Wv):
    """Build per-core in_maps (numpy only)."""
    import ml_dtypes

    bf = ml_dtypes.bfloat16

    Wkv = np.concatenate([Wk, Wv], axis=0)  # [128, E]
    wkv = np.ascontiguousarray(
        Wkv.T.reshape(8, 128, 128).transpose(1, 0, 2).reshape(128, 8 * 128)
    ).astype(bf)
    wqs = (Wq.T / np.sqrt(np.float32(D))).astype(np.float32)
    wq = np.ascontiguousarray(
        wqs.reshape(8, 128, 64).transpose(1, 0, 2).reshape(128, 8 * 64)
    ).astype(bf)

    # masks: ps2 is [kv_row_in_block (partition), q_col]; invisible = -1e30
    r = np.arange(P)
    tri = np.where(r[:, None] > r[None, :], np.float32(-1e30), np.float32(0.0))
    flat = np.full((P, P), -1e30, np.float32)
    zero = np.zeros((P, P), np.float32)
    consts = []
    for p in range(2):
        par = zero if p == 1 else flat
        # masks tile is [128, 2, 640]: per blk [mown(256)|moth(256)|mmix(128)]
        cst = np.concatenate(
            [
                np.eye(P, dtype=np.float32),      # ident8
                tri, zero, par, zero, tri,        # blk0: mown|moth|mmix
                flat, tri, flat, par, par,        # blk1: mown|moth|mmix
            ],
            axis=1,
        )
        consts.append(np.ascontiguousarray(cst).astype(bf))

    in_maps = []
    for core in range(8):
        b, p = core // 2, core % 2
        xb = x[b]
        # chunk 2j = own rows of group j; chunk 2j+1 = other rows
        blocks = xb.reshape(NPOS, P, E)
        order = []
        for j in range(4):
            order += [8 * j + 2 * t + p for t in range(4)]
            order += [8 * j + 2 * t + (1 - p) for t in range(4)]
        xb = blocks[order].reshape(S, E)
        t = xb.reshape(NCH, 512, 8, 128)  # [it, n, c, p]
        x2 = np.ascontiguousarray(t.transpose(3, 0, 2, 1)).astype(bf)
        x2 = x2.reshape(128, NCH * 8 * 512)
        # chunk 0 re-laid as two 256-row halves, each [c, 256]-major
        c0 = x2[:, 0:4096].reshape(128, 8, 2, 256)
        x2 = x2.copy()
        x2[:, 0:4096] = np.ascontiguousarray(
            c0.transpose(0, 2, 1, 3)
        ).reshape(128, 4096)
        in_maps.append({"x2": x2, "wkv": wkv, "wq": wq, "cst": consts[p]})
    return in_maps


def _assemble(results):
    out = np.empty((B, S, D), np.float32)
    for core in range(8):
        b, p = core // 2, core % 2
        y = np.asarray(results[core]["y"], dtype=np.float32).reshape(NSUP, 65, 512)
        for s in range(NSUP):
            blk = (y[s, 0:64, :] / y[s, 64:65, :]).T  # [512, 64]
            for t in range(4):
                g = 8 * s + 2 * t + p
                out[b, g * P : (g + 1) * P, :] = blk[t * 128 : (t + 1) * 128]
    return out


def _get_program():
    if "nc" not in _prog_cache:
        _prog_cache["nc"] = _build_program()
    return _prog_cache["nc"]


def run(inputs, trace=False, trace_kwargs=None):
    from concourse import bass_utils

    nc = _get_program()
    in_maps = _host_inputs(
        inputs["x"], inputs["Wq"], inputs["Wk"], inputs["Wv"]
    )
    res = bass_utils.run_bass_kernel_spmd(
        nc,
        in_maps,
        core_ids=list(range(8)),
        trace=trace,
        **(trace_kwargs or {}),
    )
    return _assemble(res.results), res


def kernel(x, Wq, Wk, Wv):
    out, _ = run({"x": x, "Wq": Wq, "Wk": Wk, "Wv": Wv})
    return out


# revision 42
# speedup vs baseline: 1.1439x; 1.1439x over previous
"""Masked causal self-attention on 8 trn2 NeuronCores (v2).

Problem: x[4,4096,1024] fp32; q/k/v = x @ W{q,k,v}.T (D=64);
out = softmax(causal(q k^T / 8)) v   -> [4, 4096, 64].

Sharding: core = (batch, parity).  Core (b,p) owns the alternating
128-row blocks {2i+p} of batch b (2048 q rows) and builds k/v for all
4096 rows.

Geometry (v2): global rows are grouped in 1024-row groups j=0..3.
Host chunk order per core: chunk 2j = the core's OWN 512 rows of group
j (global blocks 8j+p, 8j+2+p, 8j+4+p, 8j+6+p), chunk 2j+1 = the OTHER
parity's 512 rows.  kv sequence positions are chunk-major (no
permutation): pos 8j+t = chunk (2j + t//4)'s block (t%4).  q superblock
s (own rows 512s..512s+512) = exactly chunk 2s, so sup s's qT needs ONE
chunk and its first attention pair needs only chunk 2s's kv.

Boundary masks for group s against sup s's 4 q blocks (q block t at
cols 128t): own kv block t': visible cols >= 128t', diag tri at t==t';
other kv block t': for p=1 visible cols >= 128t' (full at t==t'), for
p=0 visible cols >= 128(t'+1).  Pairing: sup0 uses own-own/oth-oth
pairs (chunk0-only first pair); sups>=1 use mixed pairs (own t', oth
t') at c0=128t' with the baseline-style [tri | parity-flat] mask.

Changes vs the 116us baseline (measured ~85us, i.e. ~1.35x):
  - unnormalized [oT | sums] PSUM block per superblock is copied to
    SBUF and DMA'd out raw; the host does the divide + transpose
    (device tail shrinks from ~10us of transpose/reciprocal/scale
    chains to one copy + one DMA).
  - DMA priority discipline: the DMA engines serve active queues
    round-robin, so transfer ORDER (not queue choice) decides arrival.
    All x chunks go single-file on the sync queue in need-order;
    weights/masks/ident overlap only the x0 prefix on scalar/gpsimd.
  - sup0's first scores gate on HALF a chunk: chunk 0 is stored as two
    256-row halves and pair (0,1) is split at column 256, so emission 0
    needs only 0.5MB of x (first exp ~21us vs ~30us).  Later chunks'
    projections are injected as per-pair fillers one superblock ahead
    of need (q-proj early: it alone gates the next superblock).
  - per-chunk PSUM parking pool separate from the scores ring (a
    shared pool deadlocks via buf rotation + the in-order PE queue).
  - merged kvT SBUF tile ([kT; vT], chunk-major columns): one
    [128,512] PSUM->SBUF copy per chunk; q proj only for even chunks
    (512 cols, 2x perf mode for <=65-row stationaries).
  - fp8 paths dropped: measured ~6.5% output error previously (fp8
    noise on q/k/v/exp does NOT average down in softmax-attention).

Known headroom: scalar-engine exp stream is the ~36us floor; PE busy
~66us incl. kv projection duplicated across parity pairs.  A pair
ReduceScatter kv exchange (kernel_cc.py) is numerically exact but
measured UNPROFITABLE: ~10us fixed overhead per collective op (4 ops =
47.9us cc_op_time >> the 6.8us of PE saved); only a sub-us remote_dma
SBUF-to-SBUF path could make the exchange pay, and the Tile scheduler
cannot order consumers of remotely-written SBUF.  Startup DMA ramp
~13us and framework drain epilogue ~4.5us are the other fixed costs.
"""

import sys

sys.path.insert(0, "/opt/trn_rl_repo")

import numpy as np

B, S, E, D = 4, 4096, 1024, 64
P = 128
NCH = 8                  # x chunks of 512 rows (even=own, odd=other)
NSUP = 4                 # q superblocks, 512 own q rows each
OWN = S // 2             # own q rows per core
NPOS = S // P            # 32 kv positions (chunk-major)

_prog_cache = {}


def _build_program():
    import concourse.mybir as mybir
    from concourse import bacc, tile

    f32 = mybir.dt.float32
    bf16 = mybir.dt.bfloat16

    nc = bacc.Bacc("TRN2", target_bir_lowering=False, debug=False, num_devices=8)
    x_d = nc.dram_tensor("x2", [P, NCH * 8 * 512], bf16, kind="ExternalInput")
    wkv_d = nc.dram_tensor("wkv", [P, 8 * 128], bf16, kind="ExternalInput")
    wq_d = nc.dram_tensor("wq", [P, 8 * 64], bf16, kind="ExternalInput")
    # const block: [ident8(128) | mown(512) | moth(512) | mmix(256)]
    const_d = nc.dram_tensor("cst", [P, 1408], bf16, kind="ExternalInput")
    y_d = nc.dram_tensor("y", [NSUP * 65, 512], f32, kind="ExternalOutput")

    with tile.TileContext(nc) as tc:
        with (
            tc.tile_pool(name="const", bufs=1) as constp,
            tc.tile_pool(name="xin", bufs=NCH) as xin,
            tc.tile_pool(name="work", bufs=3) as work,
            tc.tile_pool(name="expp", bufs=6) as expp,
            tc.tile_pool(name="ps_s", bufs=2, space="PSUM") as ps_s,
            tc.tile_pool(name="park", bufs=2, space="PSUM") as park,
            tc.tile_pool(name="aux", bufs=1, space="PSUM") as aux,
            tc.tile_pool(name="ps_po", bufs=1, space="PSUM") as ps_po,
        ):
            ident8 = constp.tile([P, P], bf16, tag="ident8")
            # masks: [blk, 640] = [mown(256) | moth(256) | mmix(128)] per blk
            masks = constp.tile([P, 2, 640], bf16, tag="masks")
            mown = masks[:, :, 0:256]
            moth = masks[:, :, 256:512]
            mmix = masks[:, :, 512:640]
            wkv_sb = constp.tile([P, 8, 128], bf16, tag="wkv")
            wq_sb = constp.tile([P, 8, 64], bf16, tag="wq")
            # merged k^T (rows 0:64) and v^T (rows 64:128), chunk-major cols
            kvT_sb = constp.tile([P, S], bf16, tag="kvT")
            qT_sb = constp.tile([64, OWN], bf16, tag="qT")
            # v natural layout + ones col per kv position
            vOnes = constp.tile([P, NPOS, 65], bf16, tag="vOnes")

            # ---- DMA prefetch (all queues; gpsimd has lowest latency) ----
            def prefetch_all():
                # DMA engines serve active queues round-robin, so transfer
                # ORDER (not queue choice) decides arrival: x chunks go
                # single-file on sync in need-order; weights on scalar and
                # ident+masks on gpsimd overlap only the x0 prefix.
                g, sy, sc = nc.gpsimd, nc.sync, nc.scalar
                cr = const_d.ap()
                wr = wkv_d.ap().rearrange("p (c m) -> p c m", c=8)
                # chunk 0 is host-stored as two 256-row halves, each
                # [c, 256]-major, so the first attention emission gates on
                # only 0.5MB of x + half the chunk-0 projection
                x0 = x_d.ap()[:, 0:4096].rearrange("p (h c n) -> p h c n", h=2, c=8)
                x1 = x_d.ap()[:, 4096:8192].rearrange("p (c n) -> p c n", c=8)

                sc.dma_start(wkv_sb[:, 0:4], wr[:, 0:4])
                g.dma_start(ident8[:], cr[:, 0:128])
                sy.dma_start(x0t[:, 0, 2:5], x0[:, 0, 2:5])
                g.dma_start(x0t[:, 0, 0:2], x0[:, 0, 0:2])
                sc.dma_start(wkv_sb[:, 4:8], wr[:, 4:8])
                sy.dma_start(x0t[:, 0, 5:8], x0[:, 0, 5:8])
                sc.dma_start(
                    wq_sb[:], wq_d.ap().rearrange("p (c m) -> p c m", c=8)
                )
                sy.dma_start(x0t[:, 1, 0:4], x0[:, 1, 0:4])
                g.dma_start(
                    masks[:], cr[:, 128:1408].rearrange("p (k c) -> p k c", k=2)
                )
                sy.dma_start(x0t[:, 1, 4:8], x0[:, 1, 4:8])
                sy.dma_start(_xt(1)[:, 0:4], x1[:, 0:4])
                sy.dma_start(_xt(1)[:, 4:8], x1[:, 4:8])
                for it in range(2, NCH):
                    src = x_d.ap()[:, it * 4096 : (it + 1) * 4096].rearrange(
                        "p (c n) -> p c n", c=8
                    )
                    sy.dma_start(_xt(it)[:], src)
                nc.vector.memset(vOnes[:, :, 64], 1.0)

            x_tiles = {}
            x0t = xin.tile([P, 2, 8, 256], bf16, tag="xn", name="xn_0")

            def _xt(it):
                if it not in x_tiles:
                    x_tiles[it] = xin.tile([P, 8, 512], bf16, tag="xn", name=f"xn_{it}")
                return x_tiles[it]

            def warmup(n):
                # ramp the PE p-state while x streams in
                wt = aux.tile([P, P], bf16, tag="aux", name="warm")
                for _ in range(n):
                    nc.tensor.transpose(wt[:], ident8[:], ident8[:])

            # ---- projections ----
            # per-chunk PSUM parking (1 bank each, 2-deep: chunk c's bank is
            # reused by chunk c+2, whose kv mm is emitted after chunk c's
            # copies -- no cross-sup rotation hazards)
            kv_psum = {}
            q_psum = {}

            def proj_kv_mm(it):
                kv_psum[it] = park.tile([P, 512], f32, tag="park", name=f"pkv_{it}")
                xn = x_tiles[it]
                for c in range(8):
                    nc.tensor.matmul(
                        kv_psum[it][:], wkv_sb[:, c], xn[:, c],
                        start=(c == 0), stop=(c == 7),
                    )

            def proj_q_mm(it):
                # even chunks only: all 512 own rows of sup it//2
                q_psum[it] = aux.tile([64, 512], f32, tag="aux", name=f"pq_{it}")
                xn = x_tiles[it]
                for c in range(8):
                    nc.tensor.matmul(
                        q_psum[it][:], wq_sb[:, c], xn[:, c],
                        start=(c == 0), stop=(c == 7),
                    )

            def proj_copies(it, engine=None):
                """[k|v]^T copy for chunk it (PSUM -> SBUF)."""
                pkv = kv_psum.pop(it)
                r0 = it * 512
                if engine is not None:
                    engine.copy(kvT_sb[:, r0 : r0 + 512], pkv[:])
                else:
                    nc.vector.tensor_scalar_mul(
                        kvT_sb[:, r0 : r0 + 512], pkv[:], 1.0
                    )

            def proj_q_copy(it):
                s = it // 2
                nc.vector.tensor_scalar_mul(
                    qT_sb[:, s * 512 : (s + 1) * 512], q_psum.pop(it)[:], 1.0
                )

            def proj_chunk0_half(h):
                """kv+q proj, kvT and qT copies for chunk 0's 256-row half
                h (its own PSUM column range; first half gates emission 0)."""
                if h == 0:
                    kv_psum[0] = park.tile([P, 512], f32, tag="park", name="pkv_0")
                    q_psum[0] = aux.tile([64, 512], f32, tag="aux", name="pq_0")
                cs = slice(h * 256, h * 256 + 256)
                for c in range(8):
                    nc.tensor.matmul(
                        kv_psum[0][:, cs], wkv_sb[:, c], x0t[:, h, c],
                        start=(c == 0), stop=(c == 7),
                    )
                for c in range(8):
                    nc.tensor.matmul(
                        q_psum[0][:, cs], wq_sb[:, c], x0t[:, h, c],
                        start=(c == 0), stop=(c == 7),
                    )
                nc.scalar.copy(kvT_sb[:, cs], kv_psum[0][:, cs])
                nc.vector.tensor_scalar_mul(qT_sb[:, cs], q_psum[0][:, cs], 1.0)

            def proj_vtr(it):
                """v transposes + vOnes fill for chunk it.  Returns the
                deferred vOnes copy (DVE) so PE/DVE interleave."""
                r0 = it * 512
                pvt = aux.tile([P, 4, 64], bf16, tag="aux", name=f"pvt_{it}")
                for b in range(4):
                    nc.tensor.transpose(
                        pvt[:, b, :],
                        kvT_sb[64:128, r0 + b * 128 : r0 + (b + 1) * 128],
                        ident8[64:128, 64:128],
                    )
                def vcp():
                    nc.vector.tensor_copy(
                        vOnes[:, it * 4 : it * 4 + 4, 0:64], pvt[:]
                    )
                return vcp

            # ---- attention for one superblock ----
            # fillers: dict pair_idx -> [fns], run after that pair's exp
            def attend_sup(s, fillers, carry=None):
                from collections import deque

                qT_s = qT_sb[:, s * 512 : (s + 1) * 512]
                # emission list: (posA, posB, c0, c1, st, mask, mask_c0, mask_w)
                # st marks the first writer of po[:, c0:c1] (AV start flag)
                pairs = []
                for j in range(s):
                    for u in range(4):
                        pairs.append(
                            (8 * j + 2 * u, 8 * j + 2 * u + 1, 0, 512,
                             j == 0 and u == 0, None, 0, 0)
                        )
                if s == 0:
                    # pair (0,1) split at col 256 so emission 0 gates on
                    # chunk 0's first 256-row half only
                    pairs.append((0, 1, 0, 256, True, mown, 0, 256))
                    pairs.append((0, 1, 256, 512, False, None, 0, 0))
                    pairs.append((2, 3, 256, 512, False, mown, 256, 256))
                    pairs.append((4, 5, 0, 512, False, moth, 0, 256))
                    pairs.append((6, 7, 256, 512, False, moth, 256, 256))
                else:
                    for t in range(4):
                        pairs.append(
                            (8 * s + t, 8 * s + 4 + t, 128 * t, 512, False,
                             mmix, 128 * t, 128)
                        )
                npair = len(pairs)
                pobox = []

                def emit_av(pr, posA, posB, c0, c1, st, expT):
                    if not pobox:
                        pobox.append(
                            ps_po.tile([65, 512], f32, tag="po", name=f"po_{s}")
                        )
                    po = pobox[0]
                    for j, pos in enumerate((posA, posB)):
                        nc.tensor.matmul(
                            po[:, c0:c1], vOnes[:, pos, :], expT[:, j, c0:c1],
                            start=(st and j == 0),
                            stop=(pr == npair - 1 and j == 1),
                        )

                pend = deque()
                for pr, (posA, posB, c0, c1, st, mask, mc0, mw) in enumerate(pairs):
                    ps2 = ps_s.tile([P, 2, 512], f32, tag="sc")
                    for j, pos in enumerate((posA, posB)):
                        nc.tensor.matmul(
                            ps2[:, j, c0:c1],
                            kvT_sb[0:64, pos * 128 : pos * 128 + 128],
                            qT_s[:, c0:c1],
                            start=True, stop=True,
                        )
                    if mask is not None:
                        nc.vector.tensor_tensor(
                            ps2[:, :, mc0 : mc0 + mw], ps2[:, :, mc0 : mc0 + mw],
                            mask, mybir.AluOpType.add,
                        )
                    expT = expp.tile([P, 2, 512], bf16, tag="expT")
                    nc.scalar.activation(
                        expT[:, :, c0:c1], ps2[:, :, c0:c1],
                        mybir.ActivationFunctionType.Exp,
                    )
                    av0, av1 = c0, c1
                    if st and (c0 > 0 or c1 < 512):
                        # a start AV must cover the whole po bank (psum acc
                        # groups are bank-granular): zero-fill outside the
                        # exp window so the out-of-window columns init to 0
                        if c0 > 0:
                            nc.vector.memset(expT[:, :, 0:c0], 0.0)
                        if c1 < 512:
                            nc.vector.memset(expT[:, :, c1:512], 0.0)
                        av0, av1 = 0, 512
                    pend.append((pr, posA, posB, av0, av1, st, expT))
                    if pr == 0 and carry:
                        carry[0]()
                    if pr == 2 and carry and len(carry) > 1:
                        carry[1]()
                    if len(pend) > 3:
                        emit_av(*pend.popleft())
                    for fn in fillers.get(pr, ()):
                        fn()

                def flush_av():
                    while pend:
                        emit_av(*pend.popleft())

                def ship():
                    o_ac = work.tile([65, 512], f32, tag="oac", name=f"oac_{s}")
                    nc.vector.tensor_copy(o_ac[:], pobox[0][:])
                    nc.sync.dma_start(
                        y_d.ap()[s * 65 : (s + 1) * 65, :], o_ac[:]
                    )

                return [flush_av, ship]

            # ---- driver ----
            # sup s's ONLY start gate is qT(chunk 2s); its boundary pairs
            # (last 4) additionally need kT/vOnes of chunks 2s and 2s+1.
            # So: q-proj(2s+2)+qT copy runs EARLY inside sup s (removing
            # the qT chain from the sup transition), while the kv work of
            # chunks 2s+2/2s+3 is deferred into sup s+1's early pairs
            # where the exp stream already has backlog.
            prefetch_all()
            warmup(14)
            proj_chunk0_half(0)

            def ch_kv(c):
                return lambda: proj_kv_mm(c)

            def ch_q(c):
                return lambda: (proj_q_mm(c), proj_q_copy(c))

            def ch_fin(c):
                def go():
                    proj_copies(c)
                    proj_vtr(c)()
                    x_tiles.pop(c)
                return go

            fill = {
                0: {0: [lambda: proj_chunk0_half(1)],
                    1: [lambda: proj_vtr(0)(), ch_kv(1)],
                    2: [ch_fin(1), ch_q(2)]},
                1: {0: [ch_kv(2)], 1: [ch_fin(2), ch_q(4)],
                    2: [ch_kv(3)], 3: [ch_fin(3)]},
                2: {0: [ch_kv(4)], 1: [ch_fin(4), ch_q(6)],
                    2: [ch_kv(5)], 3: [ch_fin(5)]},
                3: {0: [ch_kv(6)], 1: [ch_fin(6)],
                    2: [ch_kv(7)], 3: [ch_fin(7)]},
            }
            carry = None
            for s in range(NSUP):
                flush = attend_sup(s, fill[s], carry)
                carry = flush
            for c in carry:
                c()

    nc.compile()
    return nc


def _host_inputs(x, Wq, Wk, Wv):
    """Build per-core in_maps (numpy only)."""
    import ml_dtypes

    bf = ml_dtypes.bfloat16

    Wkv = np.concatenate([Wk, Wv], axis=0)  # [128, E]
    wkv = np.ascontiguousarray(
        Wkv.T.reshape(8, 128, 128).transpose(1, 0, 2).reshape(128, 8 * 128)
    ).astype(bf)
    wqs = (Wq.T / np.sqrt(np.float32(D))).astype(np.float32)
    wq = np.ascontiguousarray(
        wqs.reshape(8, 128, 64).transpose(1, 0, 2).reshape(128, 8 * 64)
    ).astype(bf)

    # masks: ps2 is [kv_row_in_block (partition), q_col]; invisible = -1e30
    r = np.arange(P)
    tri = np.where(r[:, None] > r[None, :], np.float32(-1e30), np.float32(0.0))
    flat = np.full((P, P), -1e30, np.float32)
    zero = np.zeros((P, P), np.float32)
    consts = []
    for p in range(2):
        par = zero if p == 1 else flat
        # masks tile is [128, 2, 640]: per blk [mown(256)|moth(256)|mmix(128)]
        cst = np.concatenate(
            [
                np.eye(P, dtype=np.float32),      # ident8
                tri, zero, par, zero, tri,        # blk0: mown|moth|mmix
                flat, tri, flat, par, par,        # blk1: mown|moth|mmix
            ],
            axis=1,
        )
        consts.append(np.ascontiguousarray(cst).astype(bf))

    in_maps = []
    for core in range(8):
        b, p = core // 2, core % 2
        xb = x[b]
        # chunk 2j = own rows of group j; chunk 2j+1 = other rows
        blocks = xb.reshape(NPOS, P, E)
        order = []
        for j in range(4):
            order += [8 * j + 2 * t + p for t in range(4)]
            order += [8 * j + 2 * t + (1 - p) for t in range(4)]
        xb = blocks[order].reshape(S, E)
        t = xb.reshape(NCH, 512, 8, 128)  # [it, n, c, p]
        x2 = np.ascontiguousarray(t.transpose(3, 0, 2, 1)).astype(bf)
        x2 = x2.reshape(128, NCH * 8 * 512)
        # chunk 0 re-laid as two 256-row halves, each [c, 256]-major
        c0 = x2[:, 0:4096].reshape(128, 8, 2, 256)
        x2 = x2.copy()
        x2[:, 0:4096] = np.ascontiguousarray(
            c0.transpose(0, 2, 1, 3)
        ).reshape(128, 4096)
        in_maps.append({"x2": x2, "wkv": wkv, "wq": wq, "cst": consts[p]})
    return in_maps


def _assemble(results):
    out = np.empty((B, S, D), np.float32)
    for core in range(8):
        b, p = core // 2, core % 2
        y = np.asarray(results[core]["y"], dtype=np.float32).reshape(NSUP, 65, 512)
        for s in range(NSUP):
            blk = (y[s, 0:64, :] / y[s, 64:65, :]).T  # [512, 64]
            for t in range(4):
                g = 8 * s + 2 * t + p
                out[b, g * P : (g + 1) * P, :] = blk[t * 128 : (t + 1) * 128]
    return out


def _get_program():
    if "nc" not in _prog_cache:
        _prog_cache["nc"] = _build_program()
    return _prog_cache["nc"]


def run(inputs, trace=False, trace_kwargs=None):
    from concourse import bass_utils

    nc = _get_program()
    in_maps = _host_inputs(
        inputs["x"], inputs["Wq"], inputs["Wk"], inputs["Wv"]
    )
    res = bass_utils.run_bass_kernel_spmd(
        nc,
        in_maps,
        core_ids=list(range(8)),
        trace=trace,
        **(trace_kwargs or {}),
    )
    return _assemble(res.results), res


def kernel(x, Wq, Wk, Wv):
    out, _ = run({"x": x, "Wq": Wq, "Wk": Wk, "Wv": Wv})
    return out


# revision 45
# speedup vs baseline: 1.1599x; 1.0140x over previous
"""Masked causal self-attention on 8 trn2 NeuronCores (v2).

Problem: x[4,4096,1024] fp32; q/k/v = x @ W{q,k,v}.T (D=64);
out = softmax(causal(q k^T / 8)) v   -> [4, 4096, 64].

Sharding: core = (batch, parity).  Core (b,p) owns the alternating
128-row blocks {2i+p} of batch b (2048 q rows) and builds k/v for all
4096 rows.

Geometry (v2): global rows are grouped in 1024-row groups j=0..3.
Host chunk order per core: chunk 2j = the core's OWN 512 rows of group
j (global blocks 8j+p, 8j+2+p, 8j+4+p, 8j+6+p), chunk 2j+1 = the OTHER
parity's 512 rows.  kv sequence positions are chunk-major (no
permutation): pos 8j+t = chunk (2j + t//4)'s block (t%4).  q superblock
s (own rows 512s..512s+512) = exactly chunk 2s, so sup s's qT needs ONE
chunk and its first attention pair needs only chunk 2s's kv.

Boundary masks for group s against sup s's 4 q blocks (q block t at
cols 128t): own kv block t': visible cols >= 128t', diag tri at t==t';
other kv block t': for p=1 visible cols >= 128t' (full at t==t'), for
p=0 visible cols >= 128(t'+1).  Pairing: sup0 uses own-own/oth-oth
pairs (chunk0-only first pair); sups>=1 use mixed pairs (own t', oth
t') at c0=128t' with the baseline-style [tri | parity-flat] mask.

Changes vs the 116us baseline (measured ~85us, i.e. ~1.35x):
  - unnormalized [oT | sums] PSUM block per superblock is copied to
    SBUF and DMA'd out raw; the host does the divide + transpose
    (device tail shrinks from ~10us of transpose/reciprocal/scale
    chains to one copy + one DMA).
  - DMA priority discipline: the DMA engines serve active queues
    round-robin, so transfer ORDER (not queue choice) decides arrival.
    All x chunks go single-file on the sync queue in need-order;
    weights/masks/ident overlap only the x0 prefix on scalar/gpsimd.
  - sup0's first scores gate on ONE chunk (own/other chunk split per
    1024-row group); later chunks' projections are injected as
    per-pair fillers one superblock ahead of need.
  - per-chunk PSUM parking pool separate from the scores ring (a
    shared pool deadlocks via buf rotation + the in-order PE queue).
  - merged kvT SBUF tile ([kT; vT], chunk-major columns): one
    [128,512] PSUM->SBUF copy per chunk; q proj only for even chunks
    (512 cols, 2x perf mode for <=65-row stationaries).
  - fp8 paths dropped: measured ~6.5% output error previously (fp8
    noise on q/k/v/exp does NOT average down in softmax-attention).

Known headroom: scalar-engine exp stream is the ~36us floor; PE busy
~66us incl. kv projection duplicated across parity pairs.  A pair
ReduceScatter kv exchange (kernel_cc.py) is numerically exact but
measured UNPROFITABLE: ~10us fixed overhead per collective op (4 ops =
47.9us cc_op_time >> the 6.8us of PE saved); only a sub-us remote_dma
SBUF-to-SBUF path could make the exchange pay, and the Tile scheduler
cannot order consumers of remotely-written SBUF.  Startup DMA ramp
~13us and framework drain epilogue ~4.5us are the other fixed costs.
"""

import sys

sys.path.insert(0, "/opt/trn_rl_repo")

import numpy as np

B, S, E, D = 4, 4096, 1024, 64
P = 128
NCH = 8                  # x chunks of 512 rows (even=own, odd=other)
NSUP = 4                 # q superblocks, 512 own q rows each
OWN = S // 2             # own q rows per core
NPOS = S // P            # 32 kv positions (chunk-major)

_prog_cache = {}


def _build_program():
    import concourse.mybir as mybir
    from concourse import bacc, tile

    f32 = mybir.dt.float32
    bf16 = mybir.dt.bfloat16

    nc = bacc.Bacc("TRN2", target_bir_lowering=False, debug=False, num_devices=8)
    x_d = nc.dram_tensor("x2", [P, NCH * 8 * 512], bf16, kind="ExternalInput")
    wkv_d = nc.dram_tensor("wkv", [P, 8 * 128], bf16, kind="ExternalInput")
    wq_d = nc.dram_tensor("wq", [P, 8 * 64], bf16, kind="ExternalInput")
    # const block: [ident8(128) | mown(512) | moth(512) | mmix(256)]
    const_d = nc.dram_tensor("cst", [P, 1408], bf16, kind="ExternalInput")
    y_d = nc.dram_tensor("y", [NSUP * 65, 512], f32, kind="ExternalOutput")

    with tile.TileContext(nc) as tc:
        with (
            tc.tile_pool(name="const", bufs=1) as constp,
            tc.tile_pool(name="xin", bufs=NCH) as xin,
            tc.tile_pool(name="work", bufs=3) as work,
            tc.tile_pool(name="expp", bufs=6) as expp,
            tc.tile_pool(name="ps_s", bufs=2, space="PSUM") as ps_s,
            tc.tile_pool(name="park", bufs=2, space="PSUM") as park,
            tc.tile_pool(name="aux", bufs=1, space="PSUM") as aux,
            tc.tile_pool(name="ps_po", bufs=1, space="PSUM") as ps_po,
        ):
            ident8 = constp.tile([P, P], bf16, tag="ident8")
            # masks: [blk, 640] = [mown(256) | moth(256) | mmix(128)] per blk
            masks = constp.tile([P, 2, 640], bf16, tag="masks")
            mown = masks[:, :, 0:256]
            moth = masks[:, :, 256:512]
            mmix = masks[:, :, 512:640]
            wkv_sb = constp.tile([P, 8, 128], bf16, tag="wkv")
            wq_sb = constp.tile([P, 8, 64], bf16, tag="wq")
            # merged k^T (rows 0:64) and v^T (rows 64:128), chunk-major cols
            kvT_sb = constp.tile([P, S], bf16, tag="kvT")
            qT_sb = constp.tile([64, OWN], bf16, tag="qT")
            # v natural layout + ones col per kv position
            vOnes = constp.tile([P, NPOS, 65], bf16, tag="vOnes")

            # ---- DMA prefetch (all queues; gpsimd has lowest latency) ----
            def prefetch_all():
                # DMA engines serve active queues round-robin, so transfer
                # ORDER (not queue choice) decides arrival: x chunks go
                # single-file on sync in need-order; weights on scalar and
                # ident+masks on gpsimd overlap only the x0 prefix.
                g, sy, sc = nc.gpsimd, nc.sync, nc.scalar
                cr = const_d.ap()
                wr = wkv_d.ap().rearrange("p (c m) -> p c m", c=8)
                x0 = x_d.ap()[:, 0:4096].rearrange("p (c n) -> p c n", c=8)
                x1 = x_d.ap()[:, 4096:8192].rearrange("p (c n) -> p c n", c=8)

                sc.dma_start(wkv_sb[:, 0:4], wr[:, 0:4])
                g.dma_start(ident8[:], cr[:, 0:128])
                sy.dma_start(_xt(0)[:, 2:4], x0[:, 2:4])
                g.dma_start(_xt(0)[:, 0:2], x0[:, 0:2])
                sc.dma_start(wkv_sb[:, 4:8], wr[:, 4:8])
                sy.dma_start(_xt(0)[:, 4:6], x0[:, 4:6])
                sy.dma_start(_xt(0)[:, 6:8], x0[:, 6:8])
                sc.dma_start(
                    wq_sb[:], wq_d.ap().rearrange("p (c m) -> p c m", c=8)
                )
                g.dma_start(
                    masks[:], cr[:, 128:1408].rearrange("p (k c) -> p k c", k=2)
                )
                sy.dma_start(_xt(1)[:, 0:4], x1[:, 0:4])
                sy.dma_start(_xt(1)[:, 4:8], x1[:, 4:8])
                for it in range(2, NCH):
                    src = x_d.ap()[:, it * 4096 : (it + 1) * 4096].rearrange(
                        "p (c n) -> p c n", c=8
                    )
                    sy.dma_start(_xt(it)[:], src)
                nc.vector.memset(vOnes[:, :, 64], 1.0)

            x_tiles = {}

            def _xt(it):
                if it not in x_tiles:
                    x_tiles[it] = xin.tile([P, 8, 512], bf16, tag="xn", name=f"xn_{it}")
                return x_tiles[it]

            def warmup(n):
                # ramp the PE p-state while x streams in
                wt = aux.tile([P, P], bf16, tag="aux", name="warm")
                for _ in range(n):
                    nc.tensor.transpose(wt[:], ident8[:], ident8[:])

            # ---- projections ----
            # per-chunk PSUM parking (1 bank each, 2-deep: chunk c's bank is
            # reused by chunk c+2, whose kv mm is emitted after chunk c's
            # copies -- no cross-sup rotation hazards)
            kv_psum = {}
            q_psum = {}

            def proj_kv_mm(it):
                kv_psum[it] = park.tile([P, 512], f32, tag="park", name=f"pkv_{it}")
                xn = x_tiles[it]
                for c in range(8):
                    nc.tensor.matmul(
                        kv_psum[it][:], wkv_sb[:, c], xn[:, c],
                        start=(c == 0), stop=(c == 7),
                    )

            def proj_q_mm(it):
                # even chunks only: all 512 own rows of sup it//2
                q_psum[it] = aux.tile([64, 512], f32, tag="aux", name=f"pq_{it}")
                xn = x_tiles[it]
                for c in range(8):
                    nc.tensor.matmul(
                        q_psum[it][:], wq_sb[:, c], xn[:, c],
                        start=(c == 0), stop=(c == 7),
                    )

            def proj_copies(it, engine=None):
                """[k|v]^T copy for chunk it (PSUM -> SBUF)."""
                pkv = kv_psum.pop(it)
                r0 = it * 512
                if engine is not None:
                    engine.copy(kvT_sb[:, r0 : r0 + 512], pkv[:])
                else:
                    nc.vector.tensor_scalar_mul(
                        kvT_sb[:, r0 : r0 + 512], pkv[:], 1.0
                    )

            def proj_q_copy(it):
                s = it // 2
                nc.vector.tensor_scalar_mul(
                    qT_sb[:, s * 512 : (s + 1) * 512], q_psum.pop(it)[:], 1.0
                )

            def proj_vtr(it):
                """v transposes + vOnes fill for chunk it.  Returns the
                deferred vOnes copy (DVE) so PE/DVE interleave."""
                r0 = it * 512
                pvt = aux.tile([P, 4, 64], bf16, tag="aux", name=f"pvt_{it}")
                for b in range(4):
                    nc.tensor.transpose(
                        pvt[:, b, :],
                        kvT_sb[64:128, r0 + b * 128 : r0 + (b + 1) * 128],
                        ident8[64:128, 64:128],
                    )
                def vcp():
                    nc.vector.tensor_copy(
                        vOnes[:, it * 4 : it * 4 + 4, 0:64], pvt[:]
                    )
                return vcp

            # ---- attention for one superblock ----
            # fillers: dict pair_idx -> [fns], run after that pair's exp
            def attend_sup(s, fillers, carry=None):
                from collections import deque

                qT_s = qT_sb[:, s * 512 : (s + 1) * 512]
                # pair list: (posA, posB, c0, mask_off, mask_c0, mask_w);
                # mask_off indexes the masks tile's last dim (-1 = none)
                pairs = []
                for j in range(s):
                    for u in range(4):
                        pairs.append(
                            (8 * j + 2 * u, 8 * j + 2 * u + 1, 0, -1, 0, 0)
                        )
                if s == 0:
                    pairs.append((0, 1, 0, 0, 0, 256))
                    pairs.append((2, 3, 256, 0, 256, 256))
                    pairs.append((4, 5, 0, 256, 0, 256))
                    pairs.append((6, 7, 256, 256, 256, 256))
                else:
                    for t in range(4):
                        pairs.append(
                            (8 * s + t, 8 * s + 4 + t, 128 * t, 512, 128 * t, 128)
                        )
                npair = len(pairs)
                pobox = []

                def emit_av(pr, posA, posB, c0, expT):
                    if not pobox:
                        pobox.append(
                            ps_po.tile([65, 512], f32, tag="po", name=f"po_{s}")
                        )
                    po = pobox[0]
                    for j, pos in enumerate((posA, posB)):
                        nc.tensor.matmul(
                            po[:, c0:], vOnes[:, pos, :], expT[:, j, c0:],
                            start=(pr == 0 and j == 0),
                            stop=(pr == npair - 1 and j == 1),
                        )

                pend = deque()
                for pr, (posA, posB, c0, moff, mc0, mw) in enumerate(pairs):
                    ps2 = ps_s.tile([P, 2, 512], f32, tag="sc")
                    for j, pos in enumerate((posA, posB)):
                        nc.tensor.matmul(
                            ps2[:, j, c0:],
                            kvT_sb[0:64, pos * 128 : pos * 128 + 128],
                            qT_s[:, c0:],
                            start=True, stop=True,
                        )
                        if moff >= 0:
                            # per-block mask right after this block's scores:
                            # block 0's DVE mask overlaps block 1's matmul,
                            # shortening the scores->exp latency chain
                            nc.vector.tensor_tensor(
                                ps2[:, j, mc0 : mc0 + mw],
                                ps2[:, j, mc0 : mc0 + mw],
                                masks[:, j, moff : moff + mw],
                                mybir.AluOpType.add,
                            )
                    expT = expp.tile([P, 2, 512], bf16, tag="expT")
                    nc.scalar.activation(
                        expT[:, :, c0:], ps2[:, :, c0:],
                        mybir.ActivationFunctionType.Exp,
                    )
                    pend.append((pr, posA, posB, c0, expT))
                    if pr == 0 and carry:
                        carry[0]()
                    if pr == 2 and carry and len(carry) > 1:
                        carry[1]()
                    if len(pend) > 3:
                        emit_av(*pend.popleft())
                    for fn in fillers.get(pr, ()):
                        fn()

                def flush_av():
                    while pend:
                        emit_av(*pend.popleft())

                def ship():
                    o_ac = work.tile([65, 512], f32, tag="oac", name=f"oac_{s}")
                    nc.vector.tensor_copy(o_ac[:], pobox[0][:])
                    nc.sync.dma_start(
                        y_d.ap()[s * 65 : (s + 1) * 65, :], o_ac[:]
                    )

                return [flush_av, ship]

            # ---- driver ----
            # sup s's ONLY start gate is qT(chunk 2s); its boundary pairs
            # (last 4) additionally need kT/vOnes of chunks 2s and 2s+1.
            # So: q-proj(2s+2)+qT copy runs EARLY inside sup s (removing
            # the qT chain from the sup transition), while the kv work of
            # chunks 2s+2/2s+3 is deferred into sup s+1's early pairs
            # where the exp stream already has backlog.
            prefetch_all()
            warmup(14)
            proj_kv_mm(0)
            proj_q_mm(0)
            proj_copies(0, engine=nc.scalar)
            proj_q_copy(0)

            def ch_kv(c):
                return lambda: proj_kv_mm(c)

            def ch_q(c):
                return lambda: (proj_q_mm(c), proj_q_copy(c))

            def ch_fin(c):
                def go():
                    proj_copies(c)
                    proj_vtr(c)()
                    x_tiles.pop(c)
                return go

            fill = {
                0: {0: [lambda: proj_vtr(0)(), ch_kv(1)],
                    1: [ch_fin(1), ch_q(2)]},
                1: {0: [ch_kv(2)], 1: [ch_fin(2), ch_q(4)],
                    2: [ch_kv(3)], 3: [ch_fin(3)]},
                2: {0: [ch_kv(4)], 1: [ch_fin(4), ch_q(6)],
                    2: [ch_kv(5)], 3: [ch_fin(5)]},
                3: {0: [ch_kv(6)], 1: [ch_fin(6)],
                    2: [ch_kv(7)], 3: [ch_fin(7)]},
            }
            carry = None
            for s in range(NSUP):
                flush = attend_sup(s, fill[s], carry)
                carry = flush
            for c in carry:
                c()

    nc.compile()
    return nc


def _host_inputs(x, Wq, Wk, Wv):
    """Build per-core in_maps (numpy only)."""
    import ml_dtypes

    bf = ml_dtypes.bfloat16

    Wkv = np.concatenate([Wk, Wv], axis=0)  # [128, E]
    wkv = np.ascontiguousarray(
        Wkv.T.reshape(8, 128, 128).transpose(1, 0, 2).reshape(128, 8 * 128)
    ).astype(bf)
    wqs = (Wq.T / np.sqrt(np.float32(D))).astype(np.float32)
    wq = np.ascontiguousarray(
        wqs.reshape(8, 128, 64).transpose(1, 0, 2).reshape(128, 8 * 64)
    ).astype(bf)

    # masks: ps2 is [kv_row_in_block (partition), q_col]; invisible = -1e30
    r = np.arange(P)
    tri = np.where(r[:, None] > r[None, :], np.float32(-1e30), np.float32(0.0))
    flat = np.full((P, P), -1e30, np.float32)
    zero = np.zeros((P, P), np.float32)
    consts = []
    for p in range(2):
        par = zero if p == 1 else flat
        # masks tile is [128, 2, 640]: per blk [mown(256)|moth(256)|mmix(128)]
        cst = np.concatenate(
            [
                np.eye(P, dtype=np.float32),      # ident8
                tri, zero, par, zero, tri,        # blk0: mown|moth|mmix
                flat, tri, flat, par, par,        # blk1: mown|moth|mmix
            ],
            axis=1,
        )
        consts.append(np.ascontiguousarray(cst).astype(bf))

    in_maps = []
    for core in range(8):
        b, p = core // 2, core % 2
        xb = x[b]
        # chunk 2j = own rows of group j; chunk 2j+1 = other rows
        blocks = xb.reshape(NPOS, P, E)
        order = []
        for j in range(4):
            order += [8 * j + 2 * t + p for t in range(4)]
            order += [8 * j + 2 * t + (1 - p) for t in range(4)]
        xb = blocks[order].reshape(S, E)
        t = xb.reshape(NCH, 512, 8, 128)  # [it, n, c, p]
        x2 = np.ascontiguousarray(t.transpose(3, 0, 2, 1)).astype(bf)
        x2 = x2.reshape(128, NCH * 8 * 512)
        in_maps.append({"x2": x2, "wkv": wkv, "wq": wq, "cst": consts[p]})
    return in_maps


def _assemble(results):
    out = np.empty((B, S, D), np.float32)
    for core in range(8):
        b, p = core // 2, core % 2
        y = np.asarray(results[core]["y"], dtype=np.float32).reshape(NSUP, 65, 512)
        for s in range(NSUP):
            blk = (y[s, 0:64, :] / y[s, 64:65, :]).T  # [512, 64]
            for t in range(4):
                g = 8 * s + 2 * t + p
                out[b, g * P : (g + 1) * P, :] = blk[t * 128 : (t + 1) * 128]
    return out


def _get_program():
    if "nc" not in _prog_cache:
        _prog_cache["nc"] = _build_program()
    return _prog_cache["nc"]


def run(inputs, trace=False, trace_kwargs=None):
    from concourse import bass_utils

    nc = _get_program()
    in_maps = _host_inputs(
        inputs["x"], inputs["Wq"], inputs["Wk"], inputs["Wv"]
    )
    res = bass_utils.run_bass_kernel_spmd(
        nc,
        in_maps,
        core_ids=list(range(8)),
        trace=trace,
        **(trace_kwargs or {}),
    )
    return _assemble(res.results), res


def kernel(x, Wq, Wk, Wv):
    out, _ = run({"x": x, "Wq": Wq, "Wk": Wk, "Wv": Wv})
    return out
